# revision 5
# baseline (speedup 1.0000x reference)
"""Trainium2 Bass kernel for nn_ConsistencyConstraint (loss_fn).

Reference computation (B=4096, D=C*H*W=4096, NCLASS=10):
    ngrad_i = (g_i - min_i) / (max_i - min_i)          per-row min-max norm
    vn_i    = ngrad_i / max(||ngrad_i||, eps)
    sim     = vn @ vn.T
    xloss   = sum_{i<j, pred_i==pred_j} (1 - sim_ij) / B
    celoss  = mean cross-entropy(outputs, y)
    loss    = celoss + xloss

Restructuring (mathematically identical; ~1e-4 rel err vs the fp32 reference):

1. Cosine similarity is invariant to the per-row positive scale 1/(max-min),
   so vn_i = z_i / ||z_i|| with z_i = g_i - min_i (eps clamp inactive).
2. For same-class pairs: sum_{i<j in c} vn_i.vn_j = (||S_c||^2 - n_c) / 2 with
   S_c = sum_{i in c} vn_i, so
       xloss = (N_pairs - (sum_c ||S_c||^2 - B) / 2) / B.
   This replaces the O(B^2 D) similarity matmul with an O(B D NCLASS)
   one-hot matmul.
3. The min subtraction commutes with the matmul:
       S_c = sum_i wa_ic g_i  -  (sum_i wa_ic min_i) * ones(D),
   so the PE streams RAW g (as float32r, full PE rate, ~tf32 precision —
   no fp16 conversion pass needed) and the rank-1 min term is applied on
   the host from the (tiny) shipped min / wa tensors.

Per-core dataflow (512 rows = 4 chunks of 128 partitions; g streamed in
column pieces sized so the DVE min-reduce rate matches the DMA stream rate,
with a small first piece for an early start and a small last piece for a
short tail):
  - DVE:  row-min per piece (overlaps the DMA stream), combine; reciprocal;
          wa = onehot * (1/||z||) rounded to f32r.
  - ACT:  ssq = ||z||^2 in ONE pass: Square(g, bias=-min) with free-dim
          accumulate (junk main out goes to a broadcast dummy); sqrt.
          Activation tables are warmed at t~0 off a memset tile.
  - PE:   8 PSUM banks accumulate S' = Wa^T @ G (f32r) over the 4 chunks.
  - argmax/onehot, cross-entropy, bincount and the final assembly are
    O(B*NCLASS) host glue.
"""

import numpy as np

import concourse.bass as bass
import concourse.mybir as mybir
import concourse.tile as tile
from concourse import bacc
from concourse.bass_utils import run_bass_kernel_spmd

N_CORES = 8
B = 4096
D = 4096  # C*H*W = 1*64*64
NCLASS = 10
ROWS_PER_CORE = B // N_CORES  # 512
P = 128  # SBUF partitions
KCH = ROWS_PER_CORE // P  # 4 row-chunks per core
NFREE = 512  # PSUM bank width (fp32)
NCH = D // NFREE  # 8 column-chunks

F32 = mybir.dt.float32
F32R = mybir.dt.float32r
FP16 = mybir.dt.float16

# column pieces per chunk: small first piece (early DVE start), small last
# piece on the final chunk (short min tail)
PIECES = [
    [1024, 1024, 2048],
    [2048, 2048],
    [2048, 2048],
    [2048, 1536, 512],
]

# Results of the last device run (BassKernelResults) — exposed so an external
# harness can read exec_time_ns when tracing is enabled via BASS_TRACE=1.
LAST_RESULTS = None

_nc_cache = None


def _build_bass():
    """One SPMD program, identical on all 8 cores; only the data differs."""
    nc = bacc.Bacc()

    g_in = nc.dram_tensor("g", [ROWS_PER_CORE, D], F32R, kind="ExternalInput")
    oh_in = nc.dram_tensor("oh", [P, KCH * NCLASS], F32, kind="ExternalInput")

    s_out = nc.dram_tensor("S", [NCLASS, D], F32, kind="ExternalOutput")
    mn_out = nc.dram_tensor("mn", [P, KCH + 1], F32, kind="ExternalOutput")
    wa_out = nc.dram_tensor("wa", [P, KCH * NCLASS], F32, kind="ExternalOutput")

    with tile.TileContext(nc) as tc:
        with (
            tc.tile_pool(name="gpool", bufs=4) as gpool,
            tc.tile_pool(name="small", bufs=4) as small,
            tc.tile_pool(name="singles", bufs=1) as singles,
            tc.tile_pool(name="outp", bufs=1) as outp,
            tc.tile_pool(name="psum", bufs=1, space="PSUM") as psum,
        ):
            # ACT table warmups (Square / Sqrt / Copy each cost a ~1.3us
            # table load at first use — pay them at t~0 off a memset tile,
            # before any data dependencies exist).
            with tc.high_priority():
                wsrc = singles.tile([P, 1], F32)
                nc.gpsimd.memset(wsrc, 1.0)
                wsq = singles.tile([P, 1], FP16)
                nc.scalar.activation(
                    wsq, wsrc, mybir.ActivationFunctionType.Square
                )
                wsr = singles.tile([P, 1], F32)
                nc.scalar.activation(
                    wsr, wsrc, mybir.ActivationFunctionType.Sqrt
                )
                wcp = singles.tile([P, 1], F32)
                nc.scalar.copy(wcp, wsrc)

            # oh first: it is tiny but gates every wa multiply, and the DMA
            # queue is FIFO — issued after g it would land behind all 8MB.
            oh_sb = singles.tile([P, KCH * NCLASS], F32)
            nc.sync.dma_start(out=oh_sb, in_=oh_in[:, :])

            gts = []
            for k in range(KCH):
                gt = gpool.tile([P, D], F32R, tag="gt", name=f"gt{k}")
                rows = slice(k * P, (k + 1) * P)
                col = 0
                for w in PIECES[k]:
                    nc.sync.dma_start(
                        out=gt[:, col : col + w], in_=g_in[rows, col : col + w]
                    )
                    col += w
                gts.append(gt)

            s_sb = outp.tile([NCLASS, D], F32)
            mn_sb = outp.tile([P, KCH + 1], F32)
            wa_sb = outp.tile([P, KCH * NCLASS], F32R)
            junk = outp.tile([P, 1], FP16)  # broadcast sink for ACT main out

            acc = [
                psum.tile([NCLASS, NFREE], F32, tag=f"acc{n}", name=f"acc{n}")
                for n in range(NCH)
            ]

            for k in range(KCH):
                gt = gts[k]
                gf = gt.bitcast(F32)
                np_k = len(PIECES[k])

                # per-piece min (each waits only its piece's DMA)
                mnh = small.tile([P, np_k], F32, tag="mnh", name=f"mnh{k}")
                col = 0
                for j, w in enumerate(PIECES[k]):
                    nc.vector.tensor_reduce(
                        mnh[:, j : j + 1],
                        gf[:, col : col + w],
                        axis=mybir.AxisListType.X,
                        op=mybir.AluOpType.min,
                    )
                    col += w

                # everything downstream of the reduces is scheduled at high
                # priority so the tile scheduler does not defer it behind
                # later chunks' bulk reduces (which starves the PE).
                with tc.high_priority():
                    mn = mn_sb[:, k : k + 1]
                    nc.vector.tensor_reduce(
                        mn, mnh, axis=mybir.AxisListType.X, op=mybir.AluOpType.min
                    )
                    negm = small.tile([P, 1], F32, tag="negm")
                    nc.vector.tensor_scalar_mul(negm, mn, -1.0)

                    # ssq = ||g - min||^2 in one ACT pass (fp32, junk out)
                    ssq = small.tile([P, 1], F32, tag="ssq")
                    nc.scalar.activation(
                        junk.broadcast_to(gt.shape),
                        gf,
                        mybir.ActivationFunctionType.Square,
                        bias=negm,
                        accum_out=ssq,
                    )
                    u = small.tile([P, 1], F32, tag="u")
                    nc.scalar.activation(
                        u, ssq, mybir.ActivationFunctionType.Sqrt
                    )
                    rs = small.tile([P, 1], F32, tag="rs")
                    nc.vector.reciprocal(rs, u)
                    if k == 0:
                        # keep the spare mn column defined
                        nc.vector.tensor_copy(mn_sb[:, KCH : KCH + 1], ssq)

                    # wa = onehot * (1/||z||), rounded to f32r for the PE
                    wa = wa_sb[:, k * NCLASS : (k + 1) * NCLASS]
                    nc.vector.tensor_scalar_mul(
                        wa, oh_sb[:, k * NCLASS : (k + 1) * NCLASS], rs
                    )

                    for n in range(NCH):
                        nc.tensor.matmul(
                            acc[n][:, :],
                            wa,
                            gt[:, n * NFREE : (n + 1) * NFREE],
                            start=(k == 0),
                            stop=(k == KCH - 1),
                        )

            # ---- drain PSUM -> SBUF -> DRAM (copies split across engines) ----
            with tc.high_priority():
                for n in range(NCH):
                    dst = s_sb[:, n * NFREE : (n + 1) * NFREE]
                    if n % 2 == 0:
                        nc.vector.tensor_copy(dst, acc[n])
                    else:
                        nc.scalar.copy(dst, acc[n])
                    if n == NCH // 2 - 1:
                        nc.sync.dma_start(
                            out=s_out[:, : D // 2], in_=s_sb[:, : D // 2]
                        )
                nc.sync.dma_start(out=s_out[:, D // 2 :], in_=s_sb[:, D // 2 :])
                nc.sync.dma_start(out=mn_out[:, :], in_=mn_sb)
                nc.sync.dma_start(out=wa_out[:, :], in_=wa_sb.bitcast(F32))

    nc.compile()
    return nc


def kernel(**inputs) -> np.ndarray:
    global LAST_RESULTS, _nc_cache

    outputs = np.asarray(inputs["outputs"], dtype=np.float32)
    grad = np.asarray(inputs["grad"], dtype=np.float32).reshape(B, D)
    y = np.asarray(inputs["y"]).astype(np.int64)

    if _nc_cache is None:
        _nc_cache = _build_bass()
    nc = _nc_cache

    # host: predicted class -> one-hot (O(B*NCLASS), tiny)
    pred = np.argmax(outputs, axis=1)
    oh_full = (pred[:, None] == np.arange(NCLASS)[None, :]).astype(np.float32)

    in_maps = []
    for c in range(N_CORES):
        sl = slice(c * ROWS_PER_CORE, (c + 1) * ROWS_PER_CORE)
        # oh laid out [p, k*NCLASS+c] to match the per-chunk partition layout
        oh_core = (
            oh_full[sl]
            .reshape(KCH, P, NCLASS)
            .transpose(1, 0, 2)
            .reshape(P, KCH * NCLASS)
        )
        in_maps.append(
            {
                "g": np.ascontiguousarray(grad[sl]),
                "oh": np.ascontiguousarray(oh_core),
            }
        )

    res = run_bass_kernel_spmd(nc, in_maps, core_ids=list(range(N_CORES)))
    LAST_RESULTS = res
    results = res.results

    # ---- host gather / unshard ----
    s_full = np.zeros((NCLASS, D), dtype=np.float64)
    m_c = np.zeros(NCLASS, dtype=np.float64)
    for r in results:
        s_full += r["S"].astype(np.float64)
        mn = r["mn"][:, :KCH].astype(np.float64)  # [P, KCH]
        wa = r["wa"].astype(np.float64).reshape(P, KCH, NCLASS)
        # rank-1 min correction: m_c += sum_{p,k} wa[p,k,c] * mn[p,k]
        m_c += np.einsum("pkc,pk->c", wa, mn)
    s_full -= m_c[:, None]

    counts = np.bincount(pred, minlength=NCLASS).astype(np.float64)
    n_pairs = float((counts * (counts - 1) / 2).sum())
    xsum = float((s_full * s_full).sum())
    xloss = (n_pairs - (xsum - B) / 2.0) / B

    o64 = outputs.astype(np.float64)
    mo = o64.max(axis=1)
    se = np.exp(o64 - mo[:, None]).sum(axis=1)
    celoss = float((np.log(se) + mo - o64[np.arange(B), y]).sum()) / B

    return np.float32(celoss + xloss)
